# revision 21
# baseline (speedup 1.0000x reference)
"""Trainium2 Bass kernel for conv_downsample_2d (FIR anti-alias + 3x3 stride-2 conv).

Math: reference computes z = depthwise_corr(x_pad2, flip(fir4x4)) then
y = corr(z, w, stride 2) + b.  These fuse into a single 6x6 stride-2 conv:
    y[o,i,j] = sum_{c,u,v} K6[o,c,u,v] * x_pad2[c, 2i+u, 2j+v] + b[o]
    K6[o,c]  = full_conv2d(w[o,c], flip(fir))
K6 is computed on host (weights are tiny).

Mapping: pure batch data-parallel across 8 NeuronCores (one image each).
Per core, the image's output columns are split into 4 groups of 96; SBUF
x-tiles hold partition (g*32 + c) = channel c of column-group g, so a single
128-wide matmul contracts all 4 groups at once against a block-diagonal
[128,128] weight per tap.  36 taps accumulate into PSUM (float32r matmuls,
1 col/cycle), ScalarE evicts PSUM->SBUF with the bias fused in.
"""

import numpy as np

# Problem dims (hardcoded per contract)
N_BATCH = 8
C_IN = 32
C_OUT = 32
H = W = 768
KS = 6            # fused kernel size
HO = WO = 384     # output spatial
NCORES = 8

G = 4             # column groups
GW = WO // G      # 96 output cols per group
XCOLS = 2 * GW + (KS - 2)   # 196 x_pad cols per group
S_OUT = 32        # output rows per strip
S_XROWS = 2 * S_OUT + (KS - 2)  # 68 x_pad rows per strip
NSTRIPS = HO // S_OUT           # 12
RPB = 4           # output rows per PSUM block (N = RPB*GW = 384 <= 512)
NBLK = S_OUT // RPB             # 8 psum blocks per strip
NTAP = KS * KS    # 36

_CACHE = {}
# test-harness knobs (harness never touches these; kernel() works with defaults)
PROFILE = False
LAST_RESULTS = None


def _fused_weights(w: np.ndarray) -> np.ndarray:
    """K6[o,c] = full 2D convolution of w[o,c] with flip(fir4x4)."""
    k1 = np.array([1.0, 3.0, 3.0, 1.0], np.float64)
    k2d = np.outer(k1, k1)
    k2d /= k2d.sum()
    wf = k2d[::-1, ::-1]  # symmetric, but keep the flip for exactness
    w64 = w.astype(np.float64)
    # K6[u,v] += w[p,q] * wf[u-p, v-q]
    K6 = np.zeros((C_OUT, C_IN, KS, KS), np.float64)
    for p in range(3):
        for q in range(3):
            K6[:, :, p : p + 4, q : q + 4] += w64[:, :, p, q, None, None] * wf[None, None, :, :]
    return K6.astype(np.float32)


def _host_tensors(w: np.ndarray, b: np.ndarray):
    """Build block-diagonal per-tap weights [NTAP,128,128] and bias [128,1]."""
    K6 = _fused_weights(w)  # [o, c, u, v]
    WT = np.zeros((NTAP, 128, 128), np.float32)
    for g in range(G):
        # lhsT[k= g*32+c, m= g*32+o] = K6[o, c, u, v]
        blk = K6.transpose(2, 3, 1, 0).reshape(NTAP, C_IN, C_OUT)
        WT[:, g * 32 : g * 32 + 32, g * 32 : g * 32 + 32] = blk
    BIAS = np.tile(b.astype(np.float32), G).reshape(128, 1)
    return WT, BIAS


def _build_program_f32r():
    from contextlib import ExitStack

    import concourse.bacc as bacc
    import concourse.tile as tile
    from concourse import mybir

    f32 = mybir.dt.float32
    f32r = mybir.dt.float32r

    nc = bacc.Bacc(
        "TRN2",
        target_bir_lowering=False,
        debug=False,
        num_devices=NCORES,
    )
    x_d = nc.dram_tensor("x", [C_IN, H, W], f32r, kind="ExternalInput").ap()
    wt_d = nc.dram_tensor("wt", [NTAP, 128, 128], f32r, kind="ExternalInput").ap()
    bias_d = nc.dram_tensor("bias", [128, 1], f32, kind="ExternalInput").ap()
    zz_d = nc.dram_tensor("zz", [128, S_XROWS, XCOLS], f32r, kind="ExternalInput").ap()
    y_d = nc.dram_tensor("y", [C_OUT, HO, WO], f32, kind="ExternalOutput").ap()

    with tile.TileContext(nc) as tc, ExitStack() as ctx:
        wpool = ctx.enter_context(tc.tile_pool(name="wpool", bufs=1))
        xpool = ctx.enter_context(tc.tile_pool(name="xpool", bufs=1))
        opool = ctx.enter_context(tc.tile_pool(name="opool", bufs=2))
        ppool = ctx.enter_context(tc.tile_pool(name="ppool", bufs=8, space="PSUM"))

        wt_t = wpool.tile([128, NTAP, 128], f32r)
        nc.sync.dma_start(wt_t[:], wt_d.rearrange("t k m -> k t m"))
        bias_t = wpool.tile([128, 1], f32)
        nc.sync.dma_start(bias_t[:], bias_d[:])

        # two persistent ping-pong x tiles; pad columns zeroed once up front
        # (interior loads never touch them, so the zeros persist).
        xts = []
        for i in range(2):
            xt = xpool.tile([128, S_XROWS, XCOLS], f32r, tag=f"xt{i}")
            # left pad cols of group 0, right pad cols of group 3 (zeros-DMA:
            # memset can't encode f32r, and DMA f32r->f32r passes the verifier)
            nc.sync.dma_start(xt[0:32, :, 0:2], zz_d[0:32, :, 0:2])
            nc.sync.dma_start(
                xt[96:128, :, XCOLS - 2 : XCOLS],
                zz_d[96:128, :, XCOLS - 2 : XCOLS],
            )
            xts.append(xt)

        for s in range(NSTRIPS):
            xs0 = 64 * s - 2  # x row corresponding to tile row 0
            xt = xts[s % 2]

            r_lo = max(0, -xs0)                 # first valid tile row
            r_hi = min(S_XROWS, H - xs0)        # one past last valid tile row
            if r_lo > 0:
                nc.sync.dma_start(xt[:, 0:r_lo, :], zz_d[:, 0:r_lo, :])
            if r_hi < S_XROWS:
                nc.sync.dma_start(
                    xt[:, r_hi:S_XROWS, :], zz_d[:, 0 : S_XROWS - r_hi, :]
                )

            for g in range(G):
                cs0 = 192 * g - 2  # x col corresponding to tile col 0
                c_lo = max(0, -cs0)
                c_hi = min(XCOLS, W - cs0)
                nc.sync.dma_start(
                    xt[32 * g : 32 * (g + 1), r_lo:r_hi, c_lo:c_hi],
                    x_d[:, xs0 + r_lo : xs0 + r_hi, cs0 + c_lo : cs0 + c_hi],
                )

            ot = opool.tile([128, S_OUT, GW], f32)
            for half in range(2):
                pts = []
                for bi in range(NBLK // 2):
                    pt = ppool.tile([128, RPB, GW], f32, tag="pt")
                    pts.append(pt)
                for t in range(NTAP):
                    u, v = divmod(t, KS)
                    lhsT = wt_t[:, t, :]
                    for bi, pt in enumerate(pts):
                        blk = half * (NBLK // 2) + bi
                        rhs = xt[
                            :,
                            2 * RPB * blk + u : 2 * RPB * blk + u + 2 * RPB - 1 : 2,
                            v : v + 2 * GW - 1 : 2,
                        ]
                        nc.tensor.matmul(
                            pt[:], lhsT, rhs, start=(t == 0), stop=(t == NTAP - 1)
                        )
                for bi, pt in enumerate(pts):
                    blk = half * (NBLK // 2) + bi
                    nc.scalar.activation(
                        ot[:, RPB * blk : RPB * (blk + 1), :],
                        pt[:],
                        mybir.ActivationFunctionType.Identity,
                        bias=bias_t[:],
                    )

            for g in range(G):
                nc.sync.dma_start(
                    y_d[:, S_OUT * s : S_OUT * (s + 1), GW * g : GW * (g + 1)],
                    ot[32 * g : 32 * (g + 1), :, :],
                )

    nc.compile()
    return nc


# bf16 pipeline geometry (v2): smaller strips to fit the f32 staging +
# bf16 tiles side by side in SBUF.
S2_OUT = 24                      # output rows per strip
S2_XROWS = 2 * S2_OUT + (KS - 2)  # 52
N2STRIPS = HO // S2_OUT          # 16
N2BLK = S2_OUT // RPB            # 6 psum blocks per strip


def _build_program_bf16():
    """v2: fp32 x staged in SBUF, DVE-converted to bf16, bf16 matmuls
    (1 col/cycle vs fp32r's 2), software-pipelined so next-strip loads and
    conversion overlap this strip's matmuls; stores on the gpsimd queue."""
    from contextlib import ExitStack

    import concourse.bacc as bacc
    import concourse.tile as tile
    from concourse import mybir

    f32 = mybir.dt.float32
    bf16 = mybir.dt.bfloat16

    nc = bacc.Bacc(
        "TRN2", target_bir_lowering=False, debug=False, num_devices=NCORES
    )
    x_d = nc.dram_tensor("x", [C_IN, H, W], f32, kind="ExternalInput").ap()
    wt_d = nc.dram_tensor("wt", [NTAP, 128, 128], bf16, kind="ExternalInput").ap()
    bias_d = nc.dram_tensor("bias", [128, 1], f32, kind="ExternalInput").ap()
    y_d = nc.dram_tensor("y", [C_OUT, HO, WO], f32, kind="ExternalOutput").ap()

    with tile.TileContext(nc) as tc, ExitStack() as ctx:
        wpool = ctx.enter_context(tc.tile_pool(name="wpool", bufs=1))
        xpool = ctx.enter_context(tc.tile_pool(name="xpool", bufs=1))
        opool = ctx.enter_context(tc.tile_pool(name="opool", bufs=2))
        ppool = ctx.enter_context(tc.tile_pool(name="ppool", bufs=8, space="PSUM"))

        wt_t = wpool.tile([128, NTAP, 128], bf16)
        nc.sync.dma_start(wt_t[:], wt_d.rearrange("t k m -> k t m"))
        bias_t = wpool.tile([128, 1], f32)
        nc.sync.dma_start(bias_t[:], bias_d[:])

        # persistent ping-pong staging (f32) and matmul (bf16) tiles
        xfs, xbs = [], []
        for i in range(2):
            xf = xpool.tile([128, S2_XROWS, XCOLS], f32, tag=f"xf{i}")
            xb = xpool.tile([128, S2_XROWS, XCOLS], bf16, tag=f"xb{i}")
            # pad cols: zero once; interior loads never touch them
            nc.vector.memset(xf[0:32, :, 0:2], 0.0)
            nc.vector.memset(xf[96:128, :, XCOLS - 2 : XCOLS], 0.0)
            xfs.append(xf)
            xbs.append(xb)

        def issue_loads(s):
            xs0 = 48 * s - 2
            xf = xfs[s % 2]
            r_lo = max(0, -xs0)
            r_hi = min(S2_XROWS, H - xs0)
            if r_lo > 0:
                nc.vector.memset(xf[:, 0:r_lo, :], 0.0)
            if r_hi < S2_XROWS:
                nc.vector.memset(xf[:, r_hi:S2_XROWS, :], 0.0)
            for g in range(G):
                cs0 = 192 * g - 2
                c_lo = max(0, -cs0)
                c_hi = min(XCOLS, W - cs0)
                nc.sync.dma_start(
                    xf[32 * g : 32 * (g + 1), r_lo:r_hi, c_lo:c_hi],
                    x_d[:, xs0 + r_lo : xs0 + r_hi, cs0 + c_lo : cs0 + c_hi],
                )

        def issue_convert(s):
            nc.vector.tensor_copy(xbs[s % 2][:], xfs[s % 2][:])

        issue_loads(0)
        issue_convert(0)
        for s in range(N2STRIPS):
            if s + 1 < N2STRIPS:
                issue_loads(s + 1)
                issue_convert(s + 1)
            xb = xbs[s % 2]
            ot = opool.tile([128, S2_OUT, GW], f32)
            pts = []
            for bi in range(N2BLK):
                pt = ppool.tile([128, RPB, GW], f32, tag="pt")
                pts.append(pt)
            for t in range(NTAP):
                u, v = divmod(t, KS)
                lhsT = wt_t[:, t, :]
                for blk, pt in enumerate(pts):
                    rhs = xb[
                        :,
                        2 * RPB * blk + u : 2 * RPB * blk + u + 2 * RPB - 1 : 2,
                        v : v + 2 * GW - 1 : 2,
                    ]
                    nc.tensor.matmul(
                        pt[:], lhsT, rhs, start=(t == 0), stop=(t == NTAP - 1)
                    )
            for blk, pt in enumerate(pts):
                nc.scalar.activation(
                    ot[:, RPB * blk : RPB * (blk + 1), :],
                    pt[:],
                    mybir.ActivationFunctionType.Identity,
                    bias=bias_t[:],
                )
            for g in range(G):
                nc.gpsimd.dma_start(
                    y_d[:, S2_OUT * s : S2_OUT * (s + 1), GW * g : GW * (g + 1)],
                    ot[32 * g : 32 * (g + 1), :, :],
                )

    nc.compile()
    return nc


KS5 = 5
NTAP5 = KS5 * KS5


def _k5_weights(w: np.ndarray) -> np.ndarray:
    """K5[o,c] = full conv of w[o,c] with (1,2,1)x(1,2,1)/64.  The remaining
    (1,1)x(1,1) box of the FIR is applied to x on the VectorEngine."""
    q1 = np.array([1.0, 2.0, 1.0], np.float64) / 8.0
    q2d = np.outer(q1, q1)
    w64 = w.astype(np.float64)
    K5 = np.zeros((C_OUT, C_IN, KS5, KS5), np.float64)
    for p in range(3):
        for q in range(3):
            K5[:, :, p : p + 3, q : q + 3] += w64[:, :, p, q, None, None] * q2d
    return K5.astype(np.float32)


def _host_tensors_k5(w: np.ndarray, b: np.ndarray):
    K5 = _k5_weights(w)
    WT = np.zeros((NTAP5, 128, 128), np.float32)
    blk = K5.transpose(2, 3, 1, 0).reshape(NTAP5, C_IN, C_OUT)
    for g in range(G):
        WT[:, g * 32 : g * 32 + 32, g * 32 : g * 32 + 32] = blk
    BIAS = np.tile(b.astype(np.float32), G).reshape(128, 1)
    return WT, BIAS


def _build_program_k5():
    """v3: y = corr(box2(x_pad2), K5) stride 2.  DVE computes the 2x2 box sum
    (h-pass fused with the fp32->bf16 conversion), PE runs 25 bf16 taps."""
    from contextlib import ExitStack

    import concourse.bacc as bacc
    import concourse.tile as tile
    from concourse import mybir

    f32 = mybir.dt.float32
    bf16 = mybir.dt.bfloat16

    nc = bacc.Bacc(
        "TRN2", target_bir_lowering=False, debug=False, num_devices=NCORES
    )
    x_d = nc.dram_tensor("x", [C_IN, H, W], f32, kind="ExternalInput").ap()
    wt_d = nc.dram_tensor("wt", [NTAP5, 128, 128], bf16, kind="ExternalInput").ap()
    bias_d = nc.dram_tensor("bias", [128, 1], f32, kind="ExternalInput").ap()
    y_d = nc.dram_tensor("y", [C_OUT, HO, WO], f32, kind="ExternalOutput").ap()

    with tile.TileContext(nc) as tc, ExitStack() as ctx:
        wpool = ctx.enter_context(tc.tile_pool(name="wpool", bufs=1))
        xpool = ctx.enter_context(tc.tile_pool(name="xpool", bufs=1))
        opool = ctx.enter_context(tc.tile_pool(name="opool", bufs=2))
        ppool = ctx.enter_context(tc.tile_pool(name="ppool", bufs=8, space="PSUM"))

        wt_t = wpool.tile([128, NTAP5, 128], bf16)
        nc.sync.dma_start(wt_t[:], wt_d.rearrange("t k m -> k t m"))
        bias_t = wpool.tile([128, 1], f32)
        nc.sync.dma_start(bias_t[:], bias_d[:])

        xfs, xbs = [], []
        xh = xpool.tile([128, S2_XROWS, XCOLS], bf16, tag="xh")  # h-box, 1 buf
        for i in range(2):
            xf = xpool.tile([128, S2_XROWS, XCOLS], f32, tag=f"xf{i}")
            xb = xpool.tile([128, S2_XROWS, XCOLS], bf16, tag=f"xb{i}")
            nc.vector.memset(xf[0:32, :, 0:2], 0.0)
            nc.vector.memset(xf[96:128, :, XCOLS - 2 : XCOLS], 0.0)
            xfs.append(xf)
            xbs.append(xb)

        RH = S2_XROWS // 2  # 26: row-half boundary for load/box pipelining

        def issue_half_load(s, half):
            xs0 = 48 * s - 2
            xf = xfs[s % 2]
            h_lo, h_hi = (0, RH) if half == 0 else (RH, S2_XROWS)
            r_lo = max(h_lo, -xs0)
            r_hi = min(h_hi, H - xs0)
            if r_lo > h_lo:
                nc.vector.memset(xf[:, h_lo:r_lo, :], 0.0)
            if r_hi < h_hi:
                nc.vector.memset(xf[:, r_hi:h_hi, :], 0.0)
            for g in range(G):
                cs0 = 192 * g - 2
                c_lo = max(0, -cs0)
                c_hi = min(XCOLS, W - cs0)
                nc.sync.dma_start(
                    xf[32 * g : 32 * (g + 1), r_lo:r_hi, c_lo:c_hi],
                    x_d[:, xs0 + r_lo : xs0 + r_hi, cs0 + c_lo : cs0 + c_hi],
                )

        def issue_box_h(s, half):
            xf = xfs[s % 2]
            lo, hi = (0, RH) if half == 0 else (RH, S2_XROWS)
            # h-pass fused with the bf16 conversion (fp32 in, bf16 out)
            nc.vector.tensor_add(
                xh[:, lo:hi, 0 : XCOLS - 1],
                xf[:, lo:hi, 0 : XCOLS - 1],
                xf[:, lo:hi, 1:XCOLS],
            )

        def issue_box_v(s, half):
            xb = xbs[s % 2]
            lo, hi = (0, RH - 1) if half == 0 else (RH - 1, S2_XROWS - 1)
            # v-pass on bf16 (2x mode: row stride 392B keeps 4B alignment)
            nc.vector.tensor_add(
                xb[:, lo:hi, 0 : XCOLS - 1],
                xh[:, lo:hi, 0 : XCOLS - 1],
                xh[:, lo + 1 : hi + 1, 0 : XCOLS - 1],
            )

        def issue_loads_and_box(s):
            issue_half_load(s, 0)
            issue_box_h(s, 0)
            issue_half_load(s, 1)
            issue_box_h(s, 1)
            issue_box_v(s, 0)
            issue_box_v(s, 1)

        issue_loads_and_box(0)
        for s in range(N2STRIPS):
            if s + 1 < N2STRIPS:
                issue_loads_and_box(s + 1)
            xb = xbs[s % 2]
            ot = opool.tile([128, S2_OUT, GW], f32)
            pts = []
            for bi in range(N2BLK):
                pt = ppool.tile([128, RPB, GW], f32, tag="pt")
                pts.append(pt)
            for t in range(NTAP5):
                u, v = divmod(t, KS5)
                lhsT = wt_t[:, t, :]
                for blk, pt in enumerate(pts):
                    rhs = xb[
                        :,
                        2 * RPB * blk + u : 2 * RPB * blk + u + 2 * RPB - 1 : 2,
                        v : v + 2 * GW - 1 : 2,
                    ]
                    nc.tensor.matmul(
                        pt[:], lhsT, rhs, start=(t == 0), stop=(t == NTAP5 - 1)
                    )
            for blk, pt in enumerate(pts):
                nc.scalar.activation(
                    ot[:, RPB * blk : RPB * (blk + 1), :],
                    pt[:],
                    mybir.ActivationFunctionType.Identity,
                    bias=bias_t[:],
                )
            for g in range(G):
                nc.gpsimd.dma_start(
                    y_d[:, S2_OUT * s : S2_OUT * (s + 1), GW * g : GW * (g + 1)],
                    ot[32 * g : 32 * (g + 1), :, :],
                )

    nc.compile()
    return nc


# v4: asymmetric fold -> 4x5 kernel (20 taps).  DVE applies
# Bv=(1,2,1) (two v-boxes) and Bh=(1,1) (one h-box, fused with bf16
# conversion); all normalization (1/64) folded into the weights.
S3_OUT = 16
S3_XROWS = 2 * S3_OUT + 4        # 36
N3STRIPS = HO // S3_OUT          # 24
N3BLK = S3_OUT // RPB            # 4
NTAP45 = 4 * 5


def _host_tensors_k45(w: np.ndarray, b: np.ndarray):
    """K45 = w (*) [(1,1) rows x (1,2,1) cols] / 64  ->  [o,c,4,5]."""
    a2d = np.outer([1.0, 1.0], [1.0, 2.0, 1.0]).astype(np.float64) / 64.0
    w64 = w.astype(np.float64)
    K45 = np.zeros((C_OUT, C_IN, 4, 5), np.float64)
    for p in range(3):
        for q in range(3):
            K45[:, :, p : p + 2, q : q + 3] += w64[:, :, p, q, None, None] * a2d
    K45 = K45.astype(np.float32)
    WT = np.zeros((NTAP45, 128, 128), np.float32)
    blk = K45.transpose(2, 3, 1, 0).reshape(NTAP45, C_IN, C_OUT)
    for g in range(G):
        WT[:, g * 32 : g * 32 + 32, g * 32 : g * 32 + 32] = blk
    BIAS = np.tile(b.astype(np.float32), G).reshape(128, 1)
    return WT, BIAS


def _build_program_k45():
    from contextlib import ExitStack

    import concourse.bacc as bacc
    import concourse.tile as tile
    from concourse import mybir

    f32 = mybir.dt.float32
    bf16 = mybir.dt.bfloat16

    nc = bacc.Bacc(
        "TRN2", target_bir_lowering=False, debug=False, num_devices=NCORES
    )
    x_d = nc.dram_tensor("x", [C_IN, H, W], f32, kind="ExternalInput").ap()
    wt_d = nc.dram_tensor("wt", [NTAP45, 128, 128], bf16, kind="ExternalInput").ap()
    bias_d = nc.dram_tensor("bias", [128, 1], f32, kind="ExternalInput").ap()
    y_d = nc.dram_tensor("y", [C_OUT, HO, WO], f32, kind="ExternalOutput").ap()

    with tile.TileContext(nc) as tc, ExitStack() as ctx:
        wpool = ctx.enter_context(tc.tile_pool(name="wpool", bufs=1))
        xpool = ctx.enter_context(tc.tile_pool(name="xpool", bufs=1))
        opool = ctx.enter_context(tc.tile_pool(name="opool", bufs=2))
        ppool = ctx.enter_context(tc.tile_pool(name="ppool", bufs=8, space="PSUM"))

        wt_t = wpool.tile([128, NTAP45, 128], bf16)
        nc.sync.dma_start(wt_t[:], wt_d.rearrange("t k m -> k t m"))
        bias_t = wpool.tile([128, 1], f32)
        nc.sync.dma_start(bias_t[:], bias_d[:])

        xh = xpool.tile([128, S3_XROWS, XCOLS], bf16, tag="xh")
        xh2 = xpool.tile([128, S3_XROWS, XCOLS], bf16, tag="xh2")
        xfs, xbs = [], []
        for i in range(3):
            xf = xpool.tile([128, S3_XROWS, XCOLS], f32, tag=f"xf{i}")
            nc.vector.memset(xf[0:32, :, 0:2], 0.0)
            nc.vector.memset(xf[96:128, :, XCOLS - 2 : XCOLS], 0.0)
            xfs.append(xf)
        for i in range(2):
            xb = xpool.tile([128, S3_XROWS, XCOLS], bf16, tag=f"xb{i}")
            xbs.append(xb)

        def issue_loads(s):
            xs0 = 32 * s - 2
            xf = xfs[s % 3]
            r_lo = max(0, -xs0)
            r_hi = min(S3_XROWS, H - xs0)
            if r_lo > 0:
                nc.vector.memset(xf[:, 0:r_lo, :], 0.0)
            if r_hi < S3_XROWS:
                nc.vector.memset(xf[:, r_hi:S3_XROWS, :], 0.0)
            for g in range(G):
                cs0 = 192 * g - 2
                c_lo = max(0, -cs0)
                c_hi = min(XCOLS, W - cs0)
                nc.sync.dma_start(
                    xf[32 * g : 32 * (g + 1), r_lo:r_hi, c_lo:c_hi],
                    x_d[:, xs0 + r_lo : xs0 + r_hi, cs0 + c_lo : cs0 + c_hi],
                )

        def issue_box(s):
            xf, xb = xfs[s % 3], xbs[s % 2]
            # h-box fused with bf16 conversion: xh[r,c] = xf[r,c] + xf[r,c+1]
            nc.vector.tensor_add(
                xh[:, :, 0 : XCOLS - 1],
                xf[:, :, 0 : XCOLS - 1],
                xf[:, :, 1:XCOLS],
            )
            # v-box twice: (1,1)*(1,1) = (1,2,1) along rows
            nc.vector.tensor_add(
                xh2[:, 0 : S3_XROWS - 1, 0 : XCOLS - 1],
                xh[:, 0 : S3_XROWS - 1, 0 : XCOLS - 1],
                xh[:, 1:S3_XROWS, 0 : XCOLS - 1],
            )
            nc.vector.tensor_add(
                xb[:, 0 : S3_XROWS - 2, 0 : XCOLS - 1],
                xh2[:, 0 : S3_XROWS - 2, 0 : XCOLS - 1],
                xh2[:, 1 : S3_XROWS - 1, 0 : XCOLS - 1],
            )

        issue_loads(0)
        issue_loads(1)
        issue_box(0)
        for s in range(N3STRIPS):
            if s + 2 < N3STRIPS:
                issue_loads(s + 2)
            if s + 1 < N3STRIPS:
                issue_box(s + 1)
            xb = xbs[s % 2]
            ot = opool.tile([128, S3_OUT, GW], f32)
            pts = []
            for bi in range(N3BLK):
                pt = ppool.tile([128, RPB, GW], f32, tag="pt")
                pts.append(pt)
            for t in range(NTAP45):
                u, v = divmod(t, 5)
                lhsT = wt_t[:, t, :]
                for blk, pt in enumerate(pts):
                    rhs = xb[
                        :,
                        2 * RPB * blk + u : 2 * RPB * blk + u + 2 * RPB - 1 : 2,
                        v : v + 2 * GW - 1 : 2,
                    ]
                    nc.tensor.matmul(
                        pt[:], lhsT, rhs, start=(t == 0), stop=(t == NTAP45 - 1)
                    )
            for blk, pt in enumerate(pts):
                nc.scalar.activation(
                    ot[:, RPB * blk : RPB * (blk + 1), :],
                    pt[:],
                    mybir.ActivationFunctionType.Identity,
                    bias=bias_t[:],
                )
            for g in range(G):
                nc.gpsimd.dma_start(
                    y_d[:, S3_OUT * s : S3_OUT * (s + 1), GW * g : GW * (g + 1)],
                    ot[32 * g : 32 * (g + 1), :, :],
                )

    nc.compile()
    return nc


DTYPE = "k45"  # "k45"/"k5"/"bf16"/"f32r"


def kernel(x: np.ndarray, w: np.ndarray, b: np.ndarray) -> np.ndarray:
    global LAST_RESULTS
    from concourse.bass_utils import run_bass_kernel_spmd

    x = np.ascontiguousarray(x, np.float32)
    w = np.asarray(w, np.float32)
    b = np.asarray(b, np.float32)
    if DTYPE == "k45":
        WT, BIAS = _host_tensors_k45(w, b)
    elif DTYPE == "k5":
        WT, BIAS = _host_tensors_k5(w, b)
    else:
        WT, BIAS = _host_tensors(w, b)

    key = "nc_" + DTYPE
    if key not in _CACHE:
        _CACHE[key] = {
            "k45": _build_program_k45,
            "k5": _build_program_k5,
            "bf16": _build_program_bf16,
            "f32r": _build_program_f32r,
        }[DTYPE]()
    nc = _CACHE[key]

    if DTYPE in ("bf16", "k5", "k45"):
        import ml_dtypes

        WTb = WT.astype(ml_dtypes.bfloat16)
        in_maps = [
            {"x": x[n], "wt": WTb, "bias": BIAS} for n in range(N_BATCH)
        ]
        res = run_bass_kernel_spmd(nc, in_maps, list(range(NCORES)), trace=PROFILE)
        LAST_RESULTS = res
        out = np.stack([res.results[n]["y"] for n in range(N_BATCH)], axis=0)
        return out.astype(np.float32)

    ZZ = np.zeros((128, S_XROWS, XCOLS), np.float32)
    in_maps = [
        {"x": x[n], "wt": WT, "bias": BIAS, "zz": ZZ} for n in range(N_BATCH)
    ]
    res = run_bass_kernel_spmd(
        nc, in_maps, list(range(NCORES)), trace=PROFILE
    )
    LAST_RESULTS = res
    out = np.stack([res.results[n]["y"] for n in range(N_BATCH)], axis=0)
    return out.astype(np.float32)
